# revision 1
# baseline (speedup 1.0000x reference)
"""Trainium2 Bass kernel for: x + s -> LayerNorm(W) -> 2x2x2 avgpool -> exact GELU.

Input  x: (32, 32, 16, 32, 64) f32, sum_weight (1,), gamma (64,), beta (64,)
Output:   (32, 32, 8, 16, 32) f32

Math notes:
  v = x + s;  LN over last dim W: mean/var are shift-equivariant/invariant, so
  (v - mean_v) = (x - mean_x) and var_v = var_x  ==> sum_weight cancels exactly.
  ln = (x - mu) * rho * gamma + beta,  rho = rsqrt(var + eps)
  pooled[q, w'] = (1/8) [ S - gw[w'] * M4 + 4*(beta_e+beta_o)[w'] ]
    S   = sum_{r in quad} rho_r * (ga*x[r,2w'] + go*x[r,2w'+1])  (ga/go = even/odd gamma)
    M4  = sum_{r in quad} mu_r * rho_r,   gw = ga + go
  out = 0.5 * p * (1 + erf(p/sqrt(2))) = Gelu(p)

Layout: data-parallel over batch N (4 per core x 8 cores). On each core,
partition dim = the 128 (n, c) pairs; free dim = (d, h, w). All LN rows and all
pooling directions live along the free dimension, so the kernel is pure
DVE/ACT/GPSIMD elementwise + bn_stats work with fully contiguous DMA.
"""

import numpy as np

import concourse.bacc as bacc
import concourse.bass as bass
import concourse.tile as tile
from concourse import mybir
from concourse.bass_utils import run_bass_kernel_spmd

P = 128
N, C, D, H, W = 32, 32, 16, 32, 64
NCORES = 8
NPER = N // NCORES  # batches per core
EPS = 1e-5
F32 = mybir.dt.float32

# rows (d,h) per chunk = one d-pair * H = 64 rows of W=64 -> 4096 f32/partition
CHUNK_ELEMS = 2 * H * W  # 4096
NCHUNK = D // 2  # 8

# Fraction of the xr (x * rstd) pass done on GPSIMD (rest on DVE); rows of 64.
XR_GP_ROWS = 64  # all 64 rows on gpsimd
# d-pool split: columns (of 2048) handled by gpsimd
DPOOL_GP_COLS = 0
# h-pool on gpsimd?
HPOOL_GP = True


def _kernel_body(
    ctx, tc: tile.TileContext, out_ap: bass.AP, xs: bass.AP, cons: bass.AP
):
    nc = tc.nc

    singles = ctx.enter_context(tc.tile_pool(name="singles", bufs=1))
    xpool = ctx.enter_context(tc.tile_pool(name="xpool", bufs=3))
    sqpool = ctx.enter_context(tc.tile_pool(name="sqpool", bufs=2))
    workbig = ctx.enter_context(tc.tile_pool(name="workbig", bufs=2))
    work = ctx.enter_context(tc.tile_pool(name="work", bufs=2))
    small = ctx.enter_context(tc.tile_pool(name="small", bufs=3))

    # constants, broadcast to all partitions
    ga_t = singles.tile([P, 32], F32)
    go_t = singles.tile([P, 32], F32)
    gw_t = singles.tile([P, 32], F32)
    bw_t = singles.tile([P, 32], F32)
    for r, t in enumerate((ga_t, go_t, gw_t, bw_t)):
        nc.sync.dma_start(out=t[:], in_=cons[r : r + 1, :].to_broadcast((P, 32)))
    eps_t = singles.tile([P, 1], F32)
    nc.vector.memset(eps_t[:], EPS)
    inv64_t = singles.tile([P, 1], F32)
    nc.vector.memset(inv64_t[:], 1.0 / W)

    xsf = xs.rearrange("p d h w -> p (d h w)")
    outf = out_ap.rearrange("p d h w -> p d (h w)")

    for k in range(NCHUNK):
        xc = xpool.tile([P, CHUNK_ELEMS], F32, tag="xc")
        nc.sync.dma_start(
            out=xc[:], in_=xsf[:, k * CHUNK_ELEMS : (k + 1) * CHUNK_ELEMS]
        )

        # --- per-row stats: sum and sum-of-squares reductions over W ---
        xc3v = xc[:].rearrange("p (r w) -> p r w", w=W)
        sq = sqpool.tile([P, CHUNK_ELEMS], F32, tag="sq")
        nc.scalar.activation(sq[:], xc[:], mybir.ActivationFunctionType.Square)
        r1 = small.tile([P, 64], F32, tag="r1")
        nc.vector.tensor_reduce(
            out=r1[:], in_=xc3v, axis=mybir.AxisListType.X, op=mybir.AluOpType.add
        )
        r2 = small.tile([P, 64], F32, tag="r2")
        nc.vector.tensor_reduce(
            out=r2[:],
            in_=sq[:].rearrange("p (r w) -> p r w", w=W),
            axis=mybir.AxisListType.X,
            op=mybir.AluOpType.add,
        )
        # msq = r1^2; v64 = r2 - r1^2/64 (= 64*var); rstd = 1/sqrt(v64/64+eps)
        # Stats smalls go to GPSIMD: only the (port-safe) reduces/reciprocal
        # stay on DVE, so the GPSIMD xr window doesn't stall DVE TT ops.
        msq = small.tile([P, 64], F32, tag="msq")
        nc.gpsimd.tensor_mul(msq[:], r1[:], r1[:])
        m64 = small.tile([P, 64], F32, tag="m64")
        nc.gpsimd.tensor_mul(m64[:], msq[:], inv64_t[:].to_broadcast((P, 64)))
        v64 = small.tile([P, 64], F32, tag="v64")
        nc.gpsimd.tensor_sub(v64[:], r2[:], m64[:])
        rstd = small.tile([P, 64], F32, tag="rstd")
        nc.scalar.activation(
            rstd[:],
            v64[:],
            mybir.ActivationFunctionType.Sqrt,
            bias=eps_t[:],
            scale=1.0 / W,
        )
        nc.vector.reciprocal(out=rstd[:], in_=rstd[:])
        # mrs = 64 * mu * rho = r1 * rstd  (the 1/64 is folded into the gw
        # constant on the host side)
        mrs = small.tile([P, 64], F32, tag="mrs")
        nc.gpsimd.tensor_mul(mrs[:], r1[:], rstd[:])

        # --- xr = x * rstd (broadcast rstd over each row of 64) ---
        xr = workbig.tile([P, CHUNK_ELEMS], F32, tag="xr")
        xc3 = xc[:].rearrange("p (r w) -> p r w", w=W)
        xr3 = xr[:].rearrange("p (r w) -> p r w", w=W)
        g = XR_GP_ROWS
        if g > 0:
            nc.gpsimd.tensor_tensor(
                out=xr3[:, :g, :],
                in0=xc3[:, :g, :],
                in1=rstd[:, :g].unsqueeze(2).to_broadcast((P, g, W)),
                op=mybir.AluOpType.mult,
            )
        if g < 64:
            nc.vector.tensor_tensor(
                out=xr3[:, g:, :],
                in0=xc3[:, g:, :],
                in1=rstd[:, g:].unsqueeze(2).to_broadcast((P, 64 - g, W)),
                op=mybir.AluOpType.mult,
            )

        # --- d-pool: rows (dd, h) -> sum over dd ---
        xd = workbig.tile([P, H * W], F32, tag="xd")  # [P, 2048]
        xr_d = xr[:].rearrange("p (d r) -> p d r", d=2)
        c = DPOOL_GP_COLS
        if c > 0:
            nc.gpsimd.tensor_tensor(
                out=xd[:, :c],
                in0=xr_d[:, 0, :c],
                in1=xr_d[:, 1, :c],
                op=mybir.AluOpType.add,
            )
        if c < H * W:
            nc.vector.tensor_tensor(
                out=xd[:, c:],
                in0=xr_d[:, 0, c:],
                in1=xr_d[:, 1, c:],
                op=mybir.AluOpType.add,
            )

        # --- h-pool: [P, 32, 64] -> [P, 16, 64] ---
        xh = work.tile([P, 16, W], F32, tag="xh")
        xd3 = xd[:].rearrange("p (h t w) -> p h t w", t=2, w=W)
        heng = nc.gpsimd if HPOOL_GP else nc.vector
        heng.tensor_tensor(
            out=xh[:], in0=xd3[:, :, 0, :], in1=xd3[:, :, 1, :], op=mybir.AluOpType.add
        )

        # --- gamma combine: s = ga*xh_even + go*xh_odd  -> [P, 16, 32] ---
        xh4 = xh[:].rearrange("p h (v t) -> p h v t", t=2)
        t1 = work.tile([P, 16, 32], F32, tag="t1")
        nc.vector.tensor_tensor(
            out=t1[:],
            in0=xh4[:, :, :, 0],
            in1=ga_t[:].unsqueeze(1).to_broadcast((P, 16, 32)),
            op=mybir.AluOpType.mult,
        )
        t2 = work.tile([P, 16, 32], F32, tag="t2")
        nc.vector.tensor_tensor(
            out=t2[:],
            in0=xh4[:, :, :, 1],
            in1=go_t[:].unsqueeze(1).to_broadcast((P, 16, 32)),
            op=mybir.AluOpType.mult,
        )
        s = work.tile([P, 16, 32], F32, tag="s")
        nc.vector.tensor_add(s[:], t1[:], t2[:])

        # --- correction: M4 per quad, corr = gw * M4 ---
        m1 = small.tile([P, 32], F32, tag="m1")
        mrs_d = mrs[:].rearrange("p (d h) -> p d h", d=2)
        nc.gpsimd.tensor_add(m1[:], mrs_d[:, 0, :], mrs_d[:, 1, :])
        mq = small.tile([P, 16], F32, tag="mq")
        m1p = m1[:].rearrange("p (h t) -> p h t", t=2)
        nc.gpsimd.tensor_add(mq[:], m1p[:, :, 0], m1p[:, :, 1])

        corr = work.tile([P, 16, 32], F32, tag="corr")
        nc.vector.tensor_tensor(
            out=corr[:],
            in0=mq[:].unsqueeze(2).to_broadcast((P, 16, 32)),
            in1=gw_t[:].unsqueeze(1).to_broadcast((P, 16, 32)),
            op=mybir.AluOpType.mult,
        )
        pre = work.tile([P, 16, 32], F32, tag="pre")
        nc.vector.tensor_sub(pre[:], s[:], corr[:])
        pre2 = work.tile([P, 16, 32], F32, tag="pre2")
        nc.vector.tensor_tensor(
            out=pre2[:],
            in0=pre[:],
            in1=bw_t[:].unsqueeze(1).to_broadcast((P, 16, 32)),
            op=mybir.AluOpType.add,
        )

        # --- GELU(pre2 / 8) ---
        res = work.tile([P, 16 * 32], F32, tag="res")
        nc.scalar.activation(
            res[:],
            pre2[:].rearrange("p a b -> p (a b)"),
            mybir.ActivationFunctionType.Gelu,
            scale=0.125,
        )
        nc.sync.dma_start(out=outf[:, k, :], in_=res[:])


_CACHE: dict = {}


def _get_compiled():
    if "nc" not in _CACHE:
        nc = bacc.Bacc("TRN2", target_bir_lowering=False, debug=False)
        xs = nc.dram_tensor("xs", [P, D, H, W], F32, kind="ExternalInput").ap()
        cons = nc.dram_tensor("cons", [4, 32], F32, kind="ExternalInput").ap()
        out = nc.dram_tensor(
            "out", [P, D // 2, H // 2, W // 2], F32, kind="ExternalOutput"
        ).ap()
        from contextlib import ExitStack

        with tile.TileContext(nc) as tc, ExitStack() as ctx:
            _kernel_body(ctx, tc, out, xs, cons)
        nc.compile()
        _CACHE["nc"] = nc
    return _CACHE["nc"]


def _make_cons(gamma: np.ndarray, beta: np.ndarray) -> np.ndarray:
    ga = gamma[0::2].astype(np.float32)
    go = gamma[1::2].astype(np.float32)
    gw = (ga + go) / 64.0  # mrs carries an extra factor of 64
    bw = 4.0 * (beta[0::2] + beta[1::2]).astype(np.float32)
    return np.stack([ga, go, gw, bw]).astype(np.float32)


def kernel(x, sum_weight, gamma, beta, trace=False):
    del sum_weight  # cancels exactly in LayerNorm (shift invariance)
    nc = _get_compiled()
    x = np.ascontiguousarray(np.asarray(x), dtype=np.float32)
    cons = _make_cons(np.asarray(gamma), np.asarray(beta))
    in_maps = []
    for core in range(NCORES):
        shard = x[core * NPER : (core + 1) * NPER].reshape(P, D, H, W)
        in_maps.append({"xs": shard, "cons": cons})
    res = run_bass_kernel_spmd(nc, in_maps, core_ids=list(range(NCORES)), trace=trace)
    out = np.concatenate(
        [
            res.results[i]["out"].reshape(NPER, C, D // 2, H // 2, W // 2)
            for i in range(NCORES)
        ],
        axis=0,
    )
    if trace:
        return out, res
    return out


if __name__ == "__main__":
    rng = np.random.default_rng(0)
    x = rng.standard_normal((N, C, D, H, W), dtype=np.float32)
    sw = rng.standard_normal((1,)).astype(np.float32)
    gamma = rng.random((W,), dtype=np.float32)
    beta = rng.standard_normal((W,)).astype(np.float32)
    y = kernel(x, sw, gamma, beta)
    print(y.shape, y.dtype)



# revision 14
# speedup vs baseline: 1.0205x; 1.0205x over previous
"""Trainium2 Bass kernel for: x + s -> LayerNorm(W) -> 2x2x2 avgpool -> exact GELU.

Input  x: (32, 32, 16, 32, 64) f32, sum_weight (1,), gamma (64,), beta (64,)
Output:   (32, 32, 8, 16, 32) f32

Math notes:
  v = x + s;  LN over last dim W: mean/var are shift-equivariant/invariant, so
  sum_weight cancels exactly.
  ln = (x - mu) * rho * gamma + beta,  rho = rsqrt(var + eps)
  pooled[q, w'] = (1/8) [ S - mq[q]*gw[w'] + 4*(beta_e+beta_o)[w'] ]
    S  = sum_{r in quad} rho_r * (ga*x[r,2w'] + go*x[r,2w'+1])
    mq = sum_{r in quad} (64*mu_r) * rho_r,  gw = (ga + go)/64
  out = Gelu(pooled)

Implementation strategy:
  - Stats: ACT squares x with a parity-DEINTERLEAVED fp16 output layout
    [rows, 2, 32]; DVE then pair-sums at 2x perf mode (all operands 2-byte
    unit-stride) and row-reduces half-size inputs.  r1 via pair-sum (GP stt)
    + half-size DVE reduce.
  - Per-row scale xr = x * rstd on GPSIMD scalar_tensor_tensor (0.6 impl
    efficiency vs 0.42 for plain TT), fp16 deinterleaved output.
  - d-pool / h-pool / gamma-combine / beta all fp16 unit-stride at DVE 2x.
  - Smalls batched per chunk-pair (128 rows); tail batched per half (4
    chunks).  ACT only loads Square, Sqrt, Gelu tables.

Layout: data-parallel over batch N (4 per core x 8 cores). Partition dim =
128 (n, c) pairs; free dim = (d, h, w).  Chunk k = d in {2k, 2k+1}: 64 LN rows
of W=64 per partition.
"""

import numpy as np

import concourse.bacc as bacc
import concourse.bass as bass
import concourse.tile as tile
from concourse import mybir
from concourse.bass_utils import run_bass_kernel_spmd

P = 128
N, C, D, H, W = 32, 32, 16, 32, 64
NCORES = 8
NPER = N // NCORES
EPS = 1e-5
F32 = mybir.dt.float32
F16 = mybir.dt.float16

CHUNK = 2 * H * W          # 4096 elems / partition, 64 rows of 64
NCHUNK = D // 2            # 8
ROWS = 64                  # rows per chunk
ALU = mybir.AluOpType

# ---- engine assignment knobs ----
XR_DVE_CHUNKS = ()         # chunks whose xr multiply runs on DVE instead of GP
PS_DVE_CHUNKS = (2, 3, 4, 5, 6, 7)  # r1 pair-sum on DVE for these chunks
HPOOL_GP = False
T2_GP = True
CORR_GP = False


def _bcast(ap, shape):
    """Broadcast [P, n] AP to shape (P, ..., n) with stride-0 middle dims."""
    while len(ap.shape) < len(shape):
        ap = ap.unsqueeze(1)
    return ap.to_broadcast(shape)


def _kernel_body(ctx, tc: tile.TileContext, out_ap: bass.AP, xs: bass.AP,
                 cons: bass.AP):
    nc = tc.nc

    singles = ctx.enter_context(tc.tile_pool(name="singles", bufs=1))
    xpool = ctx.enter_context(tc.tile_pool(name="xpool", bufs=4))
    sqpool = ctx.enter_context(tc.tile_pool(name="sqpool", bufs=2))
    pspool = ctx.enter_context(tc.tile_pool(name="pspool", bufs=2))
    xrpool = ctx.enter_context(tc.tile_pool(name="xrpool", bufs=2))
    xdpool = ctx.enter_context(tc.tile_pool(name="xdpool", bufs=2))
    smpool = ctx.enter_context(tc.tile_pool(name="smpool", bufs=2))
    tailpool = ctx.enter_context(tc.tile_pool(name="tailpool", bufs=1))

    # --- constants ---
    ga_t = singles.tile([P, 32], F32)
    go_t = singles.tile([P, 32], F32)
    gw_t = singles.tile([P, 32], F32)
    bw_t = singles.tile([P, 32], F32)
    for r, t in enumerate((ga_t, go_t, gw_t, bw_t)):
        nc.sync.dma_start(out=t[:], in_=cons[r:r + 1, :].to_broadcast((P, 32)))
    ga16_t = singles.tile([P, 32], F16)
    nc.vector.tensor_scalar_mul(out=ga16_t[:], in0=ga_t[:], scalar1=1.0)
    go16_t = singles.tile([P, 32], F16)
    nc.vector.tensor_scalar_mul(out=go16_t[:], in0=go_t[:], scalar1=1.0)
    bw16_t = singles.tile([P, 32], F16)
    nc.vector.tensor_scalar_mul(out=bw16_t[:], in0=bw_t[:], scalar1=1.0)
    eps_t = singles.tile([P, 1], F32)
    nc.vector.memset(eps_t[:], EPS)

    xsf = xs.rearrange("p d h w -> p (d h w)")
    outf = out_ap.rearrange("p d h w -> p (d h w)")  # [P, 4096]

    # --- persistent staging ---
    # xh layout per half: [P, 4 chunks, 16 h', 64 w] fp16 (w interleaved)
    xh_half = [singles.tile([P, 4, 16, W], F16, name=f"xh{i}")
               for i in range(2)]
    rstd_p = [singles.tile([P, 2 * ROWS], F32, name=f"rstd{i}")
              for i in range(4)]
    r1_p = [singles.tile([P, 2 * ROWS], F32, name=f"r1v{i}")
            for i in range(4)]
    r2_p = [singles.tile([P, 2 * ROWS], F32, name=f"r2v{i}")
            for i in range(4)]
    mr_half = [singles.tile([P, 4 * ROWS], F32, name=f"mr{i}")
               for i in range(2)]

    def dma_in(k):
        xc = xpool.tile([P, CHUNK], F32, tag="xc")
        nc.sync.dma_start(out=xc[:], in_=xsf[:, k * CHUNK:(k + 1) * CHUNK])
        return xc

    def stats(k, xc):
        """Square (deinterleaved, ACT) + pair sums + row reduces."""
        p, kk = k // 2, k % 2
        # x viewed as [P, row, parity, w']
        x4 = xc[:].rearrange("p (r v t) -> p r t v", v=32, t=2)
        sq4 = sqpool.tile([P, ROWS, 2, 32], F16, tag="sq")
        nc.scalar.activation(sq4[:], x4,
                             mybir.ActivationFunctionType.Square)
        psq = pspool.tile([P, ROWS, 32], F16, tag="psq")
        nc.vector.tensor_tensor(out=psq[:], in0=sq4[:, :, 0, :],
                                in1=sq4[:, :, 1, :], op=ALU.add)
        nc.vector.tensor_reduce(out=r2_p[p][:, kk * ROWS:(kk + 1) * ROWS],
                                in_=psq[:], axis=mybir.AxisListType.X,
                                op=ALU.add)
        ps = pspool.tile([P, ROWS, 32], F32, tag="ps")
        if k in PS_DVE_CHUNKS:
            nc.vector.tensor_tensor(out=ps[:], in0=x4[:, :, 0, :],
                                    in1=x4[:, :, 1, :], op=ALU.add)
        else:
            nc.gpsimd.tensor_tensor(out=ps[:], in0=x4[:, :, 0, :],
                                    in1=x4[:, :, 1, :], op=ALU.add)
        nc.vector.tensor_reduce(out=r1_p[p][:, kk * ROWS:(kk + 1) * ROWS],
                                in_=ps[:], axis=mybir.AxisListType.X,
                                op=ALU.add)

    def smalls(p):
        """Stats recombination for a pair (128 rows): rstd, mr = 64*mu*rstd."""
        r1v, r2v = r1_p[p][:], r2_p[p][:]
        sqm = smpool.tile([P, 2 * ROWS], F32, tag="sqm")
        nc.gpsimd.tensor_tensor(out=sqm[:], in0=r1v, in1=r1v, op=ALU.mult)
        # v64 = r2 - sqm/64  (= 64 * var)
        v64 = smpool.tile([P, 2 * ROWS], F32, tag="v64")
        nc.vector.scalar_tensor_tensor(out=v64[:], in0=sqm[:],
                                       scalar=-1.0 / W, in1=r2v,
                                       op0=ALU.mult, op1=ALU.add)
        sd = smpool.tile([P, 2 * ROWS], F32, tag="sd")
        nc.scalar.activation(sd[:], v64[:],
                             mybir.ActivationFunctionType.Sqrt,
                             bias=eps_t[:], scale=1.0 / W)
        rt = rstd_p[p]
        nc.vector.reciprocal(out=rt[:], in_=sd[:])
        mrh = mr_half[p // 2]
        nc.vector.tensor_tensor(out=mrh[:, (p % 2) * 128:(p % 2) * 128 + 128],
                                in0=r1v, in1=rt[:], op=ALU.mult)

    def pools(k, xc):
        """xr = x*rstd (fp16), d-pool, h-pool into xh_half (interleaved w)."""
        p, kk = k // 2, k % 2
        rt = rstd_p[p][:, kk * ROWS:(kk + 1) * ROWS]  # [P, 64]
        x3 = xc[:].rearrange("p (r w) -> p r w", w=W)
        xr = xrpool.tile([P, ROWS, W], F16, tag="xr")
        rb = rt.unsqueeze(2).to_broadcast((P, ROWS, W))
        if k in XR_DVE_CHUNKS:
            nc.vector.tensor_tensor(out=xr[:], in0=x3, in1=rb, op=ALU.mult)
        else:
            nc.gpsimd.tensor_tensor(out=xr[:], in0=x3, in1=rb, op=ALU.mult)
        # d-pool: [P, 2, 2048] -> [P, 2048]
        xd = xdpool.tile([P, CHUNK // 2], F16, tag="xd")
        xr2 = xr[:].rearrange("p r w -> p (r w)").rearrange(
            "p (s f) -> p s f", s=2)
        nc.vector.tensor_tensor(out=xd[:], in0=xr2[:, 0, :], in1=xr2[:, 1, :],
                                op=ALU.add)
        # h-pool: [P, 16, 2, 64] -> xh_half[:, k%4]
        xd3 = xd[:].rearrange("p (h s w) -> p h s w", s=2, w=W)
        xho = xh_half[k // 4][:, k % 4, :, :]
        if HPOOL_GP:
            nc.gpsimd.tensor_tensor(out=xho, in0=xd3[:, :, 0, :],
                                    in1=xd3[:, :, 1, :], op=ALU.add)
        else:
            nc.vector.tensor_tensor(out=xho, in0=xd3[:, :, 0, :],
                                    in1=xd3[:, :, 1, :], op=ALU.add)

    def tail(h):
        """Gamma combine + mean correction + beta + GELU for chunks 4h..4h+3."""
        xh = xh_half[h][:]  # [P, 4, 16, 64]
        mr5 = mr_half[h][:].rearrange("p (k d q t) -> p k d q t", k=4, d=2,
                                      t=2)
        mq1 = tailpool.tile([P, 4, 2, 16], F32, tag="mq1")
        nc.vector.tensor_tensor(out=mq1[:], in0=mr5[:, :, :, :, 0],
                                in1=mr5[:, :, :, :, 1], op=ALU.add)
        mq = tailpool.tile([P, 4, 16], F32, tag="mq")
        nc.vector.tensor_tensor(out=mq[:], in0=mq1[:, :, 0, :],
                                in1=mq1[:, :, 1, :], op=ALU.add)

        # 3D views for stt ops: rows = (chunk, h') = 64
        xhf = xh.rearrange("p k h (v t) -> p (k h) v t", t=2)
        sh3 = (P, 64, 32)
        t1 = tailpool.tile([P, 64, 32], F16, tag="t1")
        nc.vector.tensor_tensor(out=t1[:], in0=xhf[:, :, :, 0],
                                in1=_bcast(ga16_t[:], sh3), op=ALU.mult)
        t2 = tailpool.tile([P, 64, 32], F16, tag="t2")
        if T2_GP:
            nc.gpsimd.tensor_tensor(out=t2[:], in0=xhf[:, :, :, 1],
                                    in1=_bcast(go16_t[:], sh3), op=ALU.mult)
        else:
            nc.vector.tensor_tensor(out=t2[:], in0=xhf[:, :, :, 1],
                                    in1=_bcast(go16_t[:], sh3), op=ALU.mult)
        s_t = tailpool.tile([P, 64, 32], F16, tag="s")
        nc.vector.tensor_tensor(out=s_t[:], in0=t1[:], in1=t2[:], op=ALU.add)
        corr = tailpool.tile([P, 64, 32], F16, tag="corr")
        mqb = mq[:].rearrange("p k h -> p (k h)").unsqueeze(2).to_broadcast(
            sh3)
        if CORR_GP:
            nc.gpsimd.tensor_tensor(out=corr[:], in0=mqb,
                                    in1=_bcast(gw_t[:], sh3), op=ALU.mult)
        else:
            nc.vector.tensor_tensor(out=corr[:], in0=mqb,
                                    in1=_bcast(gw_t[:], sh3), op=ALU.mult)
        pre = tailpool.tile([P, 64, 32], F16, tag="pre")
        nc.vector.tensor_tensor(out=pre[:], in0=s_t[:], in1=corr[:],
                                op=ALU.subtract)
        pre2 = tailpool.tile([P, 64, 32], F16, tag="pre2")
        nc.vector.tensor_tensor(out=pre2[:], in0=pre[:],
                                in1=_bcast(bw16_t[:], sh3), op=ALU.add)
        res = tailpool.tile([P, 4 * 512], F32, tag="res")
        nc.scalar.activation(res[:], pre2[:].rearrange("p a b -> p (a b)"),
                             mybir.ActivationFunctionType.Gelu, scale=0.125)
        nc.sync.dma_start(out=outf[:, h * 2048:(h + 1) * 2048], in_=res[:])

    # ---- schedule ----
    xc_t = [None] * NCHUNK
    for k in range(4):
        xc_t[k] = dma_in(k)
    stats(0, xc_t[0])
    stats(1, xc_t[1])
    for p in range(4):
        smalls(p)
        if p < 3:
            if 2 * p + 4 < NCHUNK:
                xc_t[2 * p + 4] = dma_in(2 * p + 4)
            if 2 * p + 5 < NCHUNK:
                xc_t[2 * p + 5] = dma_in(2 * p + 5)
            stats(2 * p + 2, xc_t[2 * p + 2])
            stats(2 * p + 3, xc_t[2 * p + 3])
        pools(2 * p, xc_t[2 * p])
        pools(2 * p + 1, xc_t[2 * p + 1])
        if p == 1:
            tail(0)
    tail(1)


_CACHE: dict = {}


def _get_compiled():
    if "nc" not in _CACHE:
        nc = bacc.Bacc("TRN2", target_bir_lowering=False, debug=False)
        xs = nc.dram_tensor("xs", [P, D, H, W], F32, kind="ExternalInput").ap()
        cons = nc.dram_tensor("cons", [4, 32], F32, kind="ExternalInput").ap()
        out = nc.dram_tensor(
            "out", [P, D // 2, H // 2, W // 2], F32, kind="ExternalOutput"
        ).ap()
        from contextlib import ExitStack

        with tile.TileContext(nc) as tc, ExitStack() as ctx:
            _kernel_body(ctx, tc, out, xs, cons)
        nc.compile()
        _CACHE["nc"] = nc
    return _CACHE["nc"]


def _make_cons(gamma: np.ndarray, beta: np.ndarray) -> np.ndarray:
    ga = gamma[0::2].astype(np.float64)
    go = gamma[1::2].astype(np.float64)
    # mr carries 64*mu*rstd -> fold the 1/64 into gw
    gw = (ga + go) / 64.0
    bw = 4.0 * (beta[0::2].astype(np.float64) + beta[1::2].astype(np.float64))
    return np.stack([ga, go, gw, bw]).astype(np.float32)


def kernel(x, sum_weight, gamma, beta, trace=False):
    del sum_weight  # cancels exactly in LayerNorm (shift invariance)
    nc = _get_compiled()
    x = np.ascontiguousarray(np.asarray(x), dtype=np.float32)
    cons = _make_cons(np.asarray(gamma), np.asarray(beta))
    in_maps = []
    for core in range(NCORES):
        shard = x[core * NPER:(core + 1) * NPER].reshape(P, D, H, W)
        in_maps.append({"xs": shard, "cons": cons})
    res = run_bass_kernel_spmd(nc, in_maps, core_ids=list(range(NCORES)),
                               trace=trace)
    out = np.concatenate(
        [
            res.results[i]["out"].reshape(NPER, C, D // 2, H // 2, W // 2)
            for i in range(NCORES)
        ],
        axis=0,
    )
    if trace:
        return out, res
    return out


if __name__ == "__main__":
    rng = np.random.default_rng(0)
    x = rng.standard_normal((N, C, D, H, W), dtype=np.float32)
    sw = rng.standard_normal((1,)).astype(np.float32)
    gamma = rng.random((W,), dtype=np.float32)
    beta = rng.standard_normal((W,)).astype(np.float32)
    y = kernel(x, sw, gamma, beta)
    print(y.shape, y.dtype)


# revision 15
# speedup vs baseline: 1.0480x; 1.0270x over previous
"""Trainium2 Bass kernel for: x + s -> LayerNorm(W) -> 2x2x2 avgpool -> exact GELU.

Input  x: (32, 32, 16, 32, 64) f32, sum_weight (1,), gamma (64,), beta (64,)
Output:   (32, 32, 8, 16, 32) f32

Math notes:
  v = x + s;  LN over last dim W: mean/var are shift-equivariant/invariant, so
  sum_weight cancels exactly.
  ln = (x - mu) * rho * gamma + beta,  rho = rsqrt(var + eps)
  pooled[q, w'] = (1/8) [ S - mq[q]*gw[w'] + 4*(beta_e+beta_o)[w'] ]
    S  = sum_{r in quad} rho_r * (ga*x[r,2w'] + go*x[r,2w'+1])
    mq = sum_{r in quad} (64*mu_r) * rho_r,  gw = (ga + go)/64
  out = Gelu(pooled)

Implementation strategy:
  - Stats: ACT squares x with a parity-DEINTERLEAVED fp16 output layout
    [rows, 2, 32]; DVE then pair-sums at 2x perf mode (all operands 2-byte
    unit-stride) and row-reduces half-size inputs.  r1 via pair-sum (GP stt)
    + half-size DVE reduce.
  - Per-row scale xr = x * rstd on GPSIMD scalar_tensor_tensor (0.6 impl
    efficiency vs 0.42 for plain TT), fp16 deinterleaved output.
  - d-pool / h-pool / gamma-combine / beta all fp16 unit-stride at DVE 2x.
  - Smalls batched per chunk-pair (128 rows); tail batched per half (4
    chunks).  ACT only loads Square, Sqrt, Gelu tables.

Layout: data-parallel over batch N (4 per core x 8 cores). Partition dim =
128 (n, c) pairs; free dim = (d, h, w).  Chunk k = d in {2k, 2k+1}: 64 LN rows
of W=64 per partition.
"""

import numpy as np

import concourse.bacc as bacc
import concourse.bass as bass
import concourse.tile as tile
from concourse import mybir
from concourse.bass_utils import run_bass_kernel_spmd

P = 128
N, C, D, H, W = 32, 32, 16, 32, 64
NCORES = 8
NPER = N // NCORES
EPS = 1e-5
F32 = mybir.dt.float32
F16 = mybir.dt.float16

CHUNK = 2 * H * W          # 4096 elems / partition, 64 rows of 64
NCHUNK = D // 2            # 8
ROWS = 64                  # rows per chunk
ALU = mybir.AluOpType

# ---- engine assignment knobs ----
PS_GP_CHUNKS = (0, 3, 4, 7)  # chunks with r1 pair-sum on GP (rest: direct DVE reduce)


def _bcast(ap, shape):
    """Broadcast [P, n] AP to shape (P, ..., n) with stride-0 middle dims."""
    while len(ap.shape) < len(shape):
        ap = ap.unsqueeze(1)
    return ap.to_broadcast(shape)


def _kernel_body(ctx, tc: tile.TileContext, out_ap: bass.AP, xs: bass.AP,
                 cons: bass.AP):
    nc = tc.nc

    singles = ctx.enter_context(tc.tile_pool(name="singles", bufs=1))
    xpool = ctx.enter_context(tc.tile_pool(name="xpool", bufs=4))
    sqpool = ctx.enter_context(tc.tile_pool(name="sqpool", bufs=2))
    pspool = ctx.enter_context(tc.tile_pool(name="pspool", bufs=2))
    xrpool = ctx.enter_context(tc.tile_pool(name="xrpool", bufs=2))
    xdpool = ctx.enter_context(tc.tile_pool(name="xdpool", bufs=2))
    smpool = ctx.enter_context(tc.tile_pool(name="smpool", bufs=2))
    tailpool = ctx.enter_context(tc.tile_pool(name="tailpool", bufs=1))

    # --- constants ---
    ga_t = singles.tile([P, 32], F32)
    go_t = singles.tile([P, 32], F32)
    gw_t = singles.tile([P, 32], F32)
    bw_t = singles.tile([P, 32], F32)
    for r, t in enumerate((ga_t, go_t, gw_t, bw_t)):
        nc.sync.dma_start(out=t[:], in_=cons[r:r + 1, :].to_broadcast((P, 32)))
    ga16_t = singles.tile([P, 32], F16)
    nc.vector.tensor_scalar_mul(out=ga16_t[:], in0=ga_t[:], scalar1=1.0)
    go16_t = singles.tile([P, 32], F16)
    nc.vector.tensor_scalar_mul(out=go16_t[:], in0=go_t[:], scalar1=1.0)
    bw16_t = singles.tile([P, 32], F16)
    nc.vector.tensor_scalar_mul(out=bw16_t[:], in0=bw_t[:], scalar1=1.0)
    eps_t = singles.tile([P, 1], F32)
    nc.vector.memset(eps_t[:], EPS)

    xsf = xs.rearrange("p d h w -> p (d h w)")
    outf = out_ap.rearrange("p d h w -> p (d h w)")  # [P, 4096]

    # --- persistent staging ---
    # xh layout per half: [P, 4 chunks, 16 h', 2 parity, 32 w'] fp16
    xh_half = [singles.tile([P, 4, 16, 2, 32], F16, name=f"xh{i}")
               for i in range(2)]
    rstd_p = [singles.tile([P, 2 * ROWS], F32, name=f"rstd{i}")
              for i in range(4)]
    r1_p = [singles.tile([P, 2 * ROWS], F32, name=f"r1v{i}")
            for i in range(4)]
    r2_p = [singles.tile([P, 2 * ROWS], F32, name=f"r2v{i}")
            for i in range(4)]
    mr_half = [singles.tile([P, 4 * ROWS], F32, name=f"mr{i}")
               for i in range(2)]

    def dma_in(k):
        xc = xpool.tile([P, CHUNK], F32, tag="xc")
        nc.sync.dma_start(out=xc[:], in_=xsf[:, k * CHUNK:(k + 1) * CHUNK])
        return xc

    def stats(k, xc):
        """Square (deinterleaved, ACT) + pair sums + row reduces."""
        p, kk = k // 2, k % 2
        # x viewed as [P, parity, row, w'] (parity OUTER -> contiguous halves)
        x4o = xc[:].rearrange("p (r v t) -> p t r v", v=32, t=2)
        sq4 = sqpool.tile([P, 2, ROWS, 32], F16, tag="sq")
        nc.scalar.activation(sq4[:], x4o,
                             mybir.ActivationFunctionType.Square)
        psq = pspool.tile([P, ROWS, 32], F16, tag="psq")
        nc.vector.tensor_tensor(out=psq[:], in0=sq4[:, 0, :, :],
                                in1=sq4[:, 1, :, :], op=ALU.add)
        nc.vector.tensor_reduce(out=r2_p[p][:, kk * ROWS:(kk + 1) * ROWS],
                                in_=psq[:], axis=mybir.AxisListType.X,
                                op=ALU.add)
        if k in PS_GP_CHUNKS:
            x4 = xc[:].rearrange("p (r v t) -> p r t v", v=32, t=2)
            ps = pspool.tile([P, ROWS, 32], F32, tag="ps")
            nc.gpsimd.tensor_tensor(out=ps[:], in0=x4[:, :, 0, :],
                                    in1=x4[:, :, 1, :], op=ALU.add)
            nc.vector.tensor_reduce(out=r1_p[p][:, kk * ROWS:(kk + 1) * ROWS],
                                    in_=ps[:], axis=mybir.AxisListType.X,
                                    op=ALU.add)
        else:
            x3 = xc[:].rearrange("p (r w) -> p r w", w=W)
            nc.vector.tensor_reduce(out=r1_p[p][:, kk * ROWS:(kk + 1) * ROWS],
                                    in_=x3, axis=mybir.AxisListType.X,
                                    op=ALU.add)

    def smalls(p):
        """Stats recombination for a pair (128 rows): rstd, mr = 64*mu*rstd."""
        r1v, r2v = r1_p[p][:], r2_p[p][:]
        sqm = smpool.tile([P, 2 * ROWS], F32, tag="sqm")
        nc.gpsimd.tensor_tensor(out=sqm[:], in0=r1v, in1=r1v, op=ALU.mult)
        # v64 = r2 - sqm/64  (= 64 * var)
        v64 = smpool.tile([P, 2 * ROWS], F32, tag="v64")
        nc.vector.scalar_tensor_tensor(out=v64[:], in0=sqm[:],
                                       scalar=-1.0 / W, in1=r2v,
                                       op0=ALU.mult, op1=ALU.add)
        sd = smpool.tile([P, 2 * ROWS], F32, tag="sd")
        nc.scalar.activation(sd[:], v64[:],
                             mybir.ActivationFunctionType.Sqrt,
                             bias=eps_t[:], scale=1.0 / W)
        rt = rstd_p[p]
        nc.vector.reciprocal(out=rt[:], in_=sd[:])
        mrh = mr_half[p // 2]
        nc.vector.tensor_tensor(out=mrh[:, (p % 2) * 128:(p % 2) * 128 + 128],
                                in0=r1v, in1=rt[:], op=ALU.mult)

    def pools(k, xc):
        """xr = x*rstd (fp16), d-pool, h-pool into xh_half (interleaved w)."""
        p, kk = k // 2, k % 2
        rt = rstd_p[p][:, kk * ROWS:(kk + 1) * ROWS]  # [P, 64]
        x4 = xc[:].rearrange("p (r v t) -> p r t v", v=32, t=2)
        # deinterleaved xr: [P, row, parity, w'] fp16 (GP reads any pattern)
        xr = xrpool.tile([P, ROWS, 2, 32], F16, tag="xr")
        rb = rt.unsqueeze(2).unsqueeze(3).to_broadcast((P, ROWS, 2, 32))
        nc.gpsimd.tensor_tensor(out=xr[:], in0=x4, in1=rb, op=ALU.mult)
        # d-pool: [P, 2, 2048] -> [P, 2048] (contiguous halves)
        xd = xdpool.tile([P, CHUNK // 2], F16, tag="xd")
        xr2 = xr[:].rearrange("p r t v -> p (r t v)").rearrange(
            "p (s f) -> p s f", s=2)
        nc.vector.tensor_tensor(out=xd[:], in0=xr2[:, 0, :], in1=xr2[:, 1, :],
                                op=ALU.add)
        # h-pool: [P, 16, 2, 64] -> xh_half[:, k%4]; 64 = (t, v)
        xd3 = xd[:].rearrange("p (h s f) -> p h s f", s=2, f=64)
        xho = xh_half[k // 4][:, k % 4, :, :, :].rearrange(
            "p h t v -> p h (t v)")
        nc.vector.tensor_tensor(out=xho, in0=xd3[:, :, 0, :],
                                in1=xd3[:, :, 1, :], op=ALU.add)

    def tail(h):
        """Gamma combine + mean correction + beta + GELU for chunks 4h..4h+3."""
        xh = xh_half[h][:]  # [P, 4, 16, 2, 32]
        mr5 = mr_half[h][:].rearrange("p (k d q t) -> p k d q t", k=4, d=2,
                                      t=2)
        mq1 = tailpool.tile([P, 4, 2, 16], F32, tag="mq1")
        nc.gpsimd.tensor_tensor(out=mq1[:], in0=mr5[:, :, :, :, 0],
                                in1=mr5[:, :, :, :, 1], op=ALU.add)
        mq = tailpool.tile([P, 4, 16], F32, tag="mq")
        nc.gpsimd.tensor_tensor(out=mq[:], in0=mq1[:, :, 0, :],
                                in1=mq1[:, :, 1, :], op=ALU.add)

        # unit-stride parity slices: [P, (k h), 32]
        xhf = xh.rearrange("p k h t v -> p (k h) t v")
        sh3 = (P, 64, 32)
        t1 = tailpool.tile([P, 64, 32], F16, tag="t1")
        nc.vector.tensor_tensor(out=t1[:], in0=xhf[:, :, 0, :],
                                in1=_bcast(ga16_t[:], sh3), op=ALU.mult)
        t2 = tailpool.tile([P, 64, 32], F16, tag="t2")
        nc.vector.tensor_tensor(out=t2[:], in0=xhf[:, :, 1, :],
                                in1=_bcast(go16_t[:], sh3), op=ALU.mult)
        s_t = tailpool.tile([P, 64, 32], F16, tag="s")
        nc.vector.tensor_tensor(out=s_t[:], in0=t1[:], in1=t2[:], op=ALU.add)
        corr = tailpool.tile([P, 64, 32], F16, tag="corr")
        mqb = mq[:].rearrange("p k h -> p (k h)").unsqueeze(2).to_broadcast(
            sh3)
        nc.gpsimd.tensor_tensor(out=corr[:], in0=mqb,
                                in1=_bcast(gw_t[:], sh3), op=ALU.mult)
        pre = tailpool.tile([P, 64, 32], F16, tag="pre")
        nc.vector.tensor_tensor(out=pre[:], in0=s_t[:], in1=corr[:],
                                op=ALU.subtract)
        pre2 = tailpool.tile([P, 64, 32], F16, tag="pre2")
        nc.vector.tensor_tensor(out=pre2[:], in0=pre[:],
                                in1=_bcast(bw16_t[:], sh3), op=ALU.add)
        res = tailpool.tile([P, 4 * 512], F32, tag="res")
        nc.scalar.activation(res[:], pre2[:].rearrange("p a b -> p (a b)"),
                             mybir.ActivationFunctionType.Gelu, scale=0.125)
        nc.sync.dma_start(out=outf[:, h * 2048:(h + 1) * 2048], in_=res[:])

    # ---- schedule ----
    xc_t = [None] * NCHUNK
    for k in range(4):
        xc_t[k] = dma_in(k)
    stats(0, xc_t[0])
    stats(1, xc_t[1])
    for p in range(4):
        smalls(p)
        if p < 3:
            if 2 * p + 4 < NCHUNK:
                xc_t[2 * p + 4] = dma_in(2 * p + 4)
            if 2 * p + 5 < NCHUNK:
                xc_t[2 * p + 5] = dma_in(2 * p + 5)
            stats(2 * p + 2, xc_t[2 * p + 2])
            stats(2 * p + 3, xc_t[2 * p + 3])
        pools(2 * p, xc_t[2 * p])
        pools(2 * p + 1, xc_t[2 * p + 1])
        if p == 1:
            tail(0)
    tail(1)


_CACHE: dict = {}


def _get_compiled():
    if "nc" not in _CACHE:
        nc = bacc.Bacc("TRN2", target_bir_lowering=False, debug=False)
        xs = nc.dram_tensor("xs", [P, D, H, W], F32, kind="ExternalInput").ap()
        cons = nc.dram_tensor("cons", [4, 32], F32, kind="ExternalInput").ap()
        out = nc.dram_tensor(
            "out", [P, D // 2, H // 2, W // 2], F32, kind="ExternalOutput"
        ).ap()
        from contextlib import ExitStack

        with tile.TileContext(nc) as tc, ExitStack() as ctx:
            _kernel_body(ctx, tc, out, xs, cons)
        nc.compile()
        _CACHE["nc"] = nc
    return _CACHE["nc"]


def _make_cons(gamma: np.ndarray, beta: np.ndarray) -> np.ndarray:
    ga = gamma[0::2].astype(np.float64)
    go = gamma[1::2].astype(np.float64)
    # mr carries 64*mu*rstd -> fold the 1/64 into gw
    gw = (ga + go) / 64.0
    bw = 4.0 * (beta[0::2].astype(np.float64) + beta[1::2].astype(np.float64))
    return np.stack([ga, go, gw, bw]).astype(np.float32)


def kernel(x, sum_weight, gamma, beta, trace=False):
    del sum_weight  # cancels exactly in LayerNorm (shift invariance)
    nc = _get_compiled()
    x = np.ascontiguousarray(np.asarray(x), dtype=np.float32)
    cons = _make_cons(np.asarray(gamma), np.asarray(beta))
    in_maps = []
    for core in range(NCORES):
        shard = x[core * NPER:(core + 1) * NPER].reshape(P, D, H, W)
        in_maps.append({"xs": shard, "cons": cons})
    res = run_bass_kernel_spmd(nc, in_maps, core_ids=list(range(NCORES)),
                               trace=trace)
    out = np.concatenate(
        [
            res.results[i]["out"].reshape(NPER, C, D // 2, H // 2, W // 2)
            for i in range(NCORES)
        ],
        axis=0,
    )
    if trace:
        return out, res
    return out


if __name__ == "__main__":
    rng = np.random.default_rng(0)
    x = rng.standard_normal((N, C, D, H, W), dtype=np.float32)
    sw = rng.standard_normal((1,)).astype(np.float32)
    gamma = rng.random((W,), dtype=np.float32)
    beta = rng.standard_normal((W,)).astype(np.float32)
    y = kernel(x, sw, gamma, beta)
    print(y.shape, y.dtype)


# revision 16
# speedup vs baseline: 1.1299x; 1.0781x over previous
"""Trainium2 Bass kernel for: x + s -> LayerNorm(W) -> 2x2x2 avgpool -> exact GELU.

Input  x: (32, 32, 16, 32, 64) f32, sum_weight (1,), gamma (64,), beta (64,)
Output:   (32, 32, 8, 16, 32) f32

Math notes:
  v = x + s;  LN over last dim W: mean/var are shift-equivariant/invariant, so
  sum_weight cancels exactly.
  ln = (x - mu) * rho * gamma + beta,  rho = rsqrt(var + eps)
  pooled[q, w'] = (1/8) [ S - mq[q]*gw[w'] + 4*(beta_e+beta_o)[w'] ]
    S  = sum_{r in quad} rho_r * (ga*x[r,2w'] + go*x[r,2w'+1])
    mq = sum_{r in quad} (64*mu_r) * rho_r,  gw = (ga + go)/64
  out = Gelu(pooled)

Implementation strategy:
  - Stats: ACT squares x with a parity-DEINTERLEAVED fp16 output layout
    [rows, 2, 32]; DVE then pair-sums at 2x perf mode (all operands 2-byte
    unit-stride) and row-reduces half-size inputs.  r1 via pair-sum (GP stt)
    + half-size DVE reduce.
  - Per-row scale xr = x * rstd on GPSIMD scalar_tensor_tensor (0.6 impl
    efficiency vs 0.42 for plain TT), fp16 deinterleaved output.
  - d-pool / h-pool / gamma-combine / beta all fp16 unit-stride at DVE 2x.
  - Smalls batched per chunk-pair (128 rows); tail batched per half (4
    chunks).  ACT only loads Square, Sqrt, Gelu tables.

Layout: data-parallel over batch N (4 per core x 8 cores). Partition dim =
128 (n, c) pairs; free dim = (d, h, w).  Chunk k = d in {2k, 2k+1}: 64 LN rows
of W=64 per partition.
"""

import numpy as np

import concourse.bacc as bacc
import concourse.bass as bass
import concourse.tile as tile
from concourse import mybir
from concourse.bass_utils import run_bass_kernel_spmd

P = 128
N, C, D, H, W = 32, 32, 16, 32, 64
NCORES = 8
NPER = N // NCORES
EPS = 1e-5
F32 = mybir.dt.float32
F16 = mybir.dt.float16

CHUNK = 2 * H * W          # 4096 elems / partition, 64 rows of 64
NCHUNK = D // 2            # 8
ROWS = 64                  # rows per chunk
ALU = mybir.AluOpType

# ---- engine assignment knobs ----
PS_GP_CHUNKS = (0, 3, 4, 7)  # chunks with r1 pair-sum on GP (rest: direct DVE reduce)


def _bcast(ap, shape):
    """Broadcast [P, n] AP to shape (P, ..., n) with stride-0 middle dims."""
    while len(ap.shape) < len(shape):
        ap = ap.unsqueeze(1)
    return ap.to_broadcast(shape)


def _kernel_body(ctx, tc: tile.TileContext, out_ap: bass.AP, xs: bass.AP,
                 cons: bass.AP):
    nc = tc.nc

    singles = ctx.enter_context(tc.tile_pool(name="singles", bufs=1))
    xpool = ctx.enter_context(tc.tile_pool(name="xpool", bufs=4))
    sqpool = ctx.enter_context(tc.tile_pool(name="sqpool", bufs=2))
    pspool = ctx.enter_context(tc.tile_pool(name="pspool", bufs=2))
    xrpool = ctx.enter_context(tc.tile_pool(name="xrpool", bufs=2))
    xdpool = ctx.enter_context(tc.tile_pool(name="xdpool", bufs=2))
    smpool = ctx.enter_context(tc.tile_pool(name="smpool", bufs=2))
    tailpool = ctx.enter_context(tc.tile_pool(name="tailpool", bufs=1))

    # --- constants ---
    ga_t = singles.tile([P, 32], F32)
    go_t = singles.tile([P, 32], F32)
    gw_t = singles.tile([P, 32], F32)
    bw_t = singles.tile([P, 32], F32)
    for r, t in enumerate((ga_t, go_t, gw_t, bw_t)):
        nc.sync.dma_start(out=t[:], in_=cons[r:r + 1, :].to_broadcast((P, 32)))
    ga16_t = singles.tile([P, 32], F16)
    nc.vector.tensor_scalar_mul(out=ga16_t[:], in0=ga_t[:], scalar1=1.0)
    go16_t = singles.tile([P, 32], F16)
    nc.vector.tensor_scalar_mul(out=go16_t[:], in0=go_t[:], scalar1=1.0)
    bw16_t = singles.tile([P, 32], F16)
    nc.vector.tensor_scalar_mul(out=bw16_t[:], in0=bw_t[:], scalar1=1.0)
    eps_t = singles.tile([P, 1], F32)
    nc.vector.memset(eps_t[:], EPS)

    xsf = xs.rearrange("p d h w -> p (d h w)")
    outf = out_ap.rearrange("p d h w -> p (d h w)")  # [P, 4096]

    # --- persistent staging ---
    # xh layout per half: [P, 4 chunks, 16 h', 2 parity, 32 w'] fp16
    xh_half = [singles.tile([P, 4, 16, 2, 32], F16, name=f"xh{i}")
               for i in range(2)]
    rstd_p = [singles.tile([P, 2 * ROWS], F32, name=f"rstd{i}")
              for i in range(4)]
    r1_p = [singles.tile([P, 2 * ROWS], F32, name=f"r1v{i}")
            for i in range(4)]
    r2_p = [singles.tile([P, 2 * ROWS], F32, name=f"r2v{i}")
            for i in range(4)]
    mr_half = [singles.tile([P, 4 * ROWS], F32, name=f"mr{i}")
               for i in range(2)]

    def dma_in(k):
        xc = xpool.tile([P, CHUNK], F32, tag="xc")
        nc.sync.dma_start(out=xc[:], in_=xsf[:, k * CHUNK:(k + 1) * CHUNK])
        return xc

    def stats(k, xc):
        """Square (deinterleaved, ACT) + pair sums + row reduces."""
        p, kk = k // 2, k % 2
        # x viewed as [P, parity, row, w'] (parity OUTER -> contiguous halves)
        x4o = xc[:].rearrange("p (r v t) -> p t r v", v=32, t=2)
        sq4 = sqpool.tile([P, 2, ROWS, 32], F16, tag="sq")
        nc.scalar.activation(sq4[:], x4o,
                             mybir.ActivationFunctionType.Square)
        psq = pspool.tile([P, ROWS, 32], F16, tag="psq")
        nc.vector.tensor_tensor(out=psq[:], in0=sq4[:, 0, :, :],
                                in1=sq4[:, 1, :, :], op=ALU.add)
        nc.vector.tensor_reduce(out=r2_p[p][:, kk * ROWS:(kk + 1) * ROWS],
                                in_=psq[:], axis=mybir.AxisListType.X,
                                op=ALU.add)
        if k in PS_GP_CHUNKS:
            x4 = xc[:].rearrange("p (r v t) -> p r t v", v=32, t=2)
            ps = pspool.tile([P, ROWS, 32], F32, tag="ps")
            nc.gpsimd.tensor_tensor(out=ps[:], in0=x4[:, :, 0, :],
                                    in1=x4[:, :, 1, :], op=ALU.add)
            nc.vector.tensor_reduce(out=r1_p[p][:, kk * ROWS:(kk + 1) * ROWS],
                                    in_=ps[:], axis=mybir.AxisListType.X,
                                    op=ALU.add)
        else:
            x3 = xc[:].rearrange("p (r w) -> p r w", w=W)
            nc.vector.tensor_reduce(out=r1_p[p][:, kk * ROWS:(kk + 1) * ROWS],
                                    in_=x3, axis=mybir.AxisListType.X,
                                    op=ALU.add)

    def smalls(p):
        """Stats recombination for a pair (128 rows): rstd, mr = 64*mu*rstd."""
        r1v, r2v = r1_p[p][:], r2_p[p][:]
        sqm = smpool.tile([P, 2 * ROWS], F32, tag="sqm")
        nc.vector.tensor_tensor(out=sqm[:], in0=r1v, in1=r1v, op=ALU.mult)
        # v64 = r2 - sqm/64  (= 64 * var)
        v64 = smpool.tile([P, 2 * ROWS], F32, tag="v64")
        nc.vector.scalar_tensor_tensor(out=v64[:], in0=sqm[:],
                                       scalar=-1.0 / W, in1=r2v,
                                       op0=ALU.mult, op1=ALU.add)
        sd = smpool.tile([P, 2 * ROWS], F32, tag="sd")
        nc.scalar.activation(sd[:], v64[:],
                             mybir.ActivationFunctionType.Sqrt,
                             bias=eps_t[:], scale=1.0 / W)
        rt = rstd_p[p]
        nc.vector.reciprocal(out=rt[:], in_=sd[:])
        mrh = mr_half[p // 2]
        nc.vector.tensor_tensor(out=mrh[:, (p % 2) * 128:(p % 2) * 128 + 128],
                                in0=r1v, in1=rt[:], op=ALU.mult)

    def xr_op(k, xc):
        """xr = x*rstd (fp16, deinterleaved out) on GPSIMD."""
        p, kk = k // 2, k % 2
        rt = rstd_p[p][:, kk * ROWS:(kk + 1) * ROWS]  # [P, 64]
        x4 = xc[:].rearrange("p (r v t) -> p r t v", v=32, t=2)
        xr = xrpool.tile([P, ROWS, 2, 32], F16, tag="xr")
        rb = rt.unsqueeze(2).unsqueeze(3).to_broadcast((P, ROWS, 2, 32))
        nc.gpsimd.tensor_tensor(out=xr[:], in0=x4, in1=rb, op=ALU.mult)
        return xr

    def pools(k, xr):
        """d-pool + h-pool into xh_half (DVE fp16 2x)."""
        # d-pool: [P, 2, 2048] -> [P, 2048] (contiguous halves)
        xd = xdpool.tile([P, CHUNK // 2], F16, tag="xd")
        xr2 = xr[:].rearrange("p r t v -> p (r t v)").rearrange(
            "p (s f) -> p s f", s=2)
        nc.vector.tensor_tensor(out=xd[:], in0=xr2[:, 0, :], in1=xr2[:, 1, :],
                                op=ALU.add)
        # h-pool: [P, 16, 2, 64] -> xh_half[:, k%4]; 64 = (t, v)
        xd3 = xd[:].rearrange("p (h s f) -> p h s f", s=2, f=64)
        xho = xh_half[k // 4][:, k % 4, :, :, :].rearrange(
            "p h t v -> p h (t v)")
        nc.vector.tensor_tensor(out=xho, in0=xd3[:, :, 0, :],
                                in1=xd3[:, :, 1, :], op=ALU.add)

    corr_t = [None, None]

    def tail_gp(h):
        """Early GP part of the tail: mean-correction term (needs only mr)."""
        mr5 = mr_half[h][:].rearrange("p (k d q t) -> p k d q t", k=4, d=2,
                                      t=2)
        mq1 = tailpool.tile([P, 4, 2, 16], F32, tag="mq1")
        nc.gpsimd.tensor_tensor(out=mq1[:], in0=mr5[:, :, :, :, 0],
                                in1=mr5[:, :, :, :, 1], op=ALU.add)
        mq = tailpool.tile([P, 4, 16], F32, tag="mq")
        nc.gpsimd.tensor_tensor(out=mq[:], in0=mq1[:, :, 0, :],
                                in1=mq1[:, :, 1, :], op=ALU.add)
        sh3 = (P, 64, 32)
        corr = tailpool.tile([P, 64, 32], F16, tag="corr")
        mqb = mq[:].rearrange("p k h -> p (k h)").unsqueeze(2).to_broadcast(
            sh3)
        nc.gpsimd.tensor_tensor(out=corr[:], in0=mqb,
                                in1=_bcast(gw_t[:], sh3), op=ALU.mult)
        corr_t[h] = corr

    def tail_dve(h):
        """Gamma combine + beta + GELU for chunks 4h..4h+3."""
        xh = xh_half[h][:]  # [P, 4, 16, 2, 32]
        # unit-stride parity slices: [P, (k h), 32]
        xhf = xh.rearrange("p k h t v -> p (k h) t v")
        sh3 = (P, 64, 32)
        t1 = tailpool.tile([P, 64, 32], F16, tag="t1")
        nc.vector.tensor_tensor(out=t1[:], in0=xhf[:, :, 0, :],
                                in1=_bcast(ga16_t[:], sh3), op=ALU.mult)
        t2 = tailpool.tile([P, 64, 32], F16, tag="t2")
        nc.vector.tensor_tensor(out=t2[:], in0=xhf[:, :, 1, :],
                                in1=_bcast(go16_t[:], sh3), op=ALU.mult)
        s_t = tailpool.tile([P, 64, 32], F16, tag="s")
        nc.vector.tensor_tensor(out=s_t[:], in0=t1[:], in1=t2[:], op=ALU.add)
        corr = corr_t[h]
        pre = tailpool.tile([P, 64, 32], F16, tag="pre")
        nc.vector.tensor_tensor(out=pre[:], in0=s_t[:], in1=corr[:],
                                op=ALU.subtract)
        pre2 = tailpool.tile([P, 64, 32], F16, tag="pre2")
        nc.vector.tensor_tensor(out=pre2[:], in0=pre[:],
                                in1=_bcast(bw16_t[:], sh3), op=ALU.add)
        res = tailpool.tile([P, 4 * 512], F32, tag="res")
        nc.scalar.activation(res[:], pre2[:].rearrange("p a b -> p (a b)"),
                             mybir.ActivationFunctionType.Gelu, scale=0.125)
        nc.sync.dma_start(out=outf[:, h * 2048:(h + 1) * 2048], in_=res[:])

    # ---- schedule ----
    xc_t = [None] * NCHUNK
    for k in range(4):
        xc_t[k] = dma_in(k)
    stats(0, xc_t[0])
    stats(1, xc_t[1])
    for p in range(4):
        smalls(p)
        xr_a = xr_op(2 * p, xc_t[2 * p])
        xr_b = xr_op(2 * p + 1, xc_t[2 * p + 1])
        if p == 2:
            tail_gp(0)
        if p == 3:
            tail_gp(1)
        if p < 3:
            if 2 * p + 4 < NCHUNK:
                xc_t[2 * p + 4] = dma_in(2 * p + 4)
            if 2 * p + 5 < NCHUNK:
                xc_t[2 * p + 5] = dma_in(2 * p + 5)
            stats(2 * p + 2, xc_t[2 * p + 2])
            stats(2 * p + 3, xc_t[2 * p + 3])
        pools(2 * p, xr_a)
        pools(2 * p + 1, xr_b)
        if p == 2:
            tail_dve(0)
    tail_dve(1)


_CACHE: dict = {}


def _get_compiled():
    if "nc" not in _CACHE:
        nc = bacc.Bacc("TRN2", target_bir_lowering=False, debug=False)
        xs = nc.dram_tensor("xs", [P, D, H, W], F32, kind="ExternalInput").ap()
        cons = nc.dram_tensor("cons", [4, 32], F32, kind="ExternalInput").ap()
        out = nc.dram_tensor(
            "out", [P, D // 2, H // 2, W // 2], F32, kind="ExternalOutput"
        ).ap()
        from contextlib import ExitStack

        with tile.TileContext(nc) as tc, ExitStack() as ctx:
            _kernel_body(ctx, tc, out, xs, cons)
        nc.compile()
        _CACHE["nc"] = nc
    return _CACHE["nc"]


def _make_cons(gamma: np.ndarray, beta: np.ndarray) -> np.ndarray:
    ga = gamma[0::2].astype(np.float64)
    go = gamma[1::2].astype(np.float64)
    # mr carries 64*mu*rstd -> fold the 1/64 into gw
    gw = (ga + go) / 64.0
    bw = 4.0 * (beta[0::2].astype(np.float64) + beta[1::2].astype(np.float64))
    return np.stack([ga, go, gw, bw]).astype(np.float32)


def kernel(x, sum_weight, gamma, beta, trace=False):
    del sum_weight  # cancels exactly in LayerNorm (shift invariance)
    nc = _get_compiled()
    x = np.ascontiguousarray(np.asarray(x), dtype=np.float32)
    cons = _make_cons(np.asarray(gamma), np.asarray(beta))
    in_maps = []
    for core in range(NCORES):
        shard = x[core * NPER:(core + 1) * NPER].reshape(P, D, H, W)
        in_maps.append({"xs": shard, "cons": cons})
    res = run_bass_kernel_spmd(nc, in_maps, core_ids=list(range(NCORES)),
                               trace=trace)
    out = np.concatenate(
        [
            res.results[i]["out"].reshape(NPER, C, D // 2, H // 2, W // 2)
            for i in range(NCORES)
        ],
        axis=0,
    )
    if trace:
        return out, res
    return out


if __name__ == "__main__":
    rng = np.random.default_rng(0)
    x = rng.standard_normal((N, C, D, H, W), dtype=np.float32)
    sw = rng.standard_normal((1,)).astype(np.float32)
    gamma = rng.random((W,), dtype=np.float32)
    beta = rng.standard_normal((W,)).astype(np.float32)
    y = kernel(x, sw, gamma, beta)
    print(y.shape, y.dtype)
